# revision 12
# baseline (speedup 1.0000x reference)
"""Trainium2 Bass kernel for ComputeNodeAreaFromRouteMap (DREAMPlace-style
weighted-overlap map sampling).

area_i = sum_{a,b} ovx[i,a] * ovy[i,b] * U[bx0_i+a, by0_i+b]

Strategy (gather-free): the per-node window lookup is the bottleneck on
TRN2 — the SWDGE dma_gather ucode costs ~2.5 ns/index engine-serially
(max 1024 idx/call), a ~330 us floor for 1M nodes.  Instead the host
BUCKETS nodes by their (qx, hy) = (bx0>>2, by0>>1) window record and
makes record identity STRUCTURAL: each of the 32768 records owns a
fixed group of KCAP=4 node slots per core, laid out so SBUF partition
p = qx and column group c>>2 = hy.  A record's nodes are dealt
round-robin across the 8 cores (capacity 32 nodes/record); the ~1% of
nodes in hotter records go to a small overflow tier whose 80-byte
records the host embeds directly in the input stream.  Leftover slots
hold size-0 dummies whose clamp-difference weights vanish.

Device work per core is then pure static-AP dense math over
131072 + 10240 slots, no per-node indirection at all:
  - window table WT[r] = U[4qx:4qx+7, 2hy:2hy+5] * BSX*BSY (b-major
    5x8 fp16 record, x-tap 7 zero-padded), streamed sequentially,
    record r at [partition r>>8, cols (r&255)*40).
  - weights: fl = pos/bs - base, fh = (pos+size)/bs - base (f32 in,
    fp16 out) where base_x = 4p (per-partition const) and base_y =
    2*(c>>2) (host map); tap weights ov[t] = clamp(fh-t,0,1) -
    clamp(fl-t,0,1).  No floor() on device: bucketing fixed the window
    base, and out-of-window taps auto-zero.
  - reduce per residue j = slot c%4 (so the record operand is a clean
    contiguous [p, c1, 5, 8] view): m = T*ovx, t = sum_a m,
    tt = t*ovy, area = sum_b tt.
Data-parallel over slots across the 8 NeuronCores; the table is
replicated.  Host work is bucketing/permutation and table layout; all
floating-point area math runs on device from raw f32 inputs.
"""
import numpy as np

import concourse.bacc as bacc
import concourse.bass as bass
import concourse.tile as tile
import concourse.mybir as mybir
from concourse import bass_utils

# ---- problem constants (hardcoded per the task contract) ----
XL, YL, XH, YH = 0.0, 0.0, 1000.0, 1000.0
NUM_MOVABLE = 1_000_000
NBX, NBY = 512, 512
BSX = (XH - XL) / NBX            # 1.953125
BSY = (YH - YL) / NBY
INV_BSX = 1.0 / BSX
INV_BSY = 1.0 / BSY

NCORES = 8
P = 128                          # partitions == qx blocks
NHY = 256                        # hy values == column groups
KCAP = 4                         # node slots per record per core
NPP = NHY * KCAP                 # 1024 main slot columns per partition
NPC = P * NPP                    # 131072 main slots per core
NREC = P * NHY                   # 32768 window records
NTAPX = 8                        # x taps 0..6 + zero pad (record a-dim)
NTAPY = 5                        # y taps 0..4
ESIZE = NTAPX * NTAPY            # 40 fp16 elems per record (b-major)
NCHUNK = 4                       # column-group chunks per pass
CHY = NHY // NCHUNK              # 64 hy groups per chunk
OVC = 80                         # overflow slot columns per partition
NOV = P * OVC                    # 10240 overflow slots per core
NPPO = NPP + OVC                 # output columns per partition

f32 = mybir.dt.float32
f16 = mybir.dt.float16

AL = mybir.AluOpType
AX = mybir.AxisListType


def build(repeat=1, num_cores=NCORES):
    nc = bacc.Bacc(None, target_bir_lowering=False, debug=False)

    x_in = nc.dram_tensor("x_in", [NPC], f32, kind="ExternalInput")
    y_in = nc.dram_tensor("y_in", [NPC], f32, kind="ExternalInput")
    sx_in = nc.dram_tensor("sx_in", [NPC], f32, kind="ExternalInput")
    sy_in = nc.dram_tensor("sy_in", [NPC], f32, kind="ExternalInput")
    wt_in = nc.dram_tensor("wt_in", [NREC * ESIZE], f16, kind="ExternalInput")
    bx_in = nc.dram_tensor("bx_in", [P], f32, kind="ExternalInput")
    by_in = nc.dram_tensor("by_in", [P * NPP], f32, kind="ExternalInput")
    ov_in = nc.dram_tensor("ov_in", [P * OVC * 6], f32, kind="ExternalInput")
    orec_in = nc.dram_tensor("orec_in", [P * OVC * ESIZE], f16,
                             kind="ExternalInput")
    area_out = nc.dram_tensor("area_out", [P * NPPO], f32,
                              kind="ExternalOutput")

    x_t = x_in[:].rearrange("(p c) -> p c", p=P)
    y_t = y_in[:].rearrange("(p c) -> p c", p=P)
    sx_t = sx_in[:].rearrange("(p c) -> p c", p=P)
    sy_t = sy_in[:].rearrange("(p c) -> p c", p=P)
    wt_t = wt_in[:].rearrange("(p c) -> p c", p=P)
    by_t = by_in[:].rearrange("(p c) -> p c", p=P)
    ov_t = ov_in[:].rearrange("(p c) -> p c", p=P)
    orec_t = orec_in[:].rearrange("(p c) -> p c", p=P)
    out_t = area_out[:].rearrange("(p c) -> p c", p=P)

    with tile.TileContext(nc) as tc:
        with (
            tc.tile_pool(name="const", bufs=1) as cpool,
            tc.tile_pool(name="inp", bufs=1) as xpool,
            tc.tile_pool(name="scr", bufs=1) as spool,
            tc.tile_pool(name="per", bufs=1) as gpool,
            tc.tile_pool(name="wts", bufs=2) as wpool,
            tc.tile_pool(name="tbl", bufs=3) as tpool,
            tc.tile_pool(name="red", bufs=2) as rpool,
            tc.tile_pool(name="out", bufs=2) as opool,
        ):
            iotax = cpool.tile([P, NTAPX], f16)
            for k in range(NTAPX):
                nc.vector.memset(iotax[:, k:k + 1], float(k))
            bxc = cpool.tile([P, 1], f32)

            def weights(v, fl, fh, ncols, ntap, pool, tag):
                """ov[t] = clamp(fh-t,0,1) - clamp(fl-t,0,1): [P,ncols,ntap]
                fl/fh: [P, ncols] APs."""
                ov = pool.tile([P, ncols, ntap], f16, tag=f"{tag}ov")
                d2 = spool.tile([P, ncols, ntap], f16, tag=f"{tag}d2")
                iota_b = iotax[:, 0:ntap].unsqueeze(1).to_broadcast(
                    [P, ncols, ntap])
                v.tensor_tensor(ov[:], fh.unsqueeze(2).to_broadcast(
                    [P, ncols, ntap]), iota_b, AL.subtract)
                v.tensor_scalar(ov[:], ov[:], 0.0, 1.0, AL.max, AL.min)
                v.tensor_tensor(d2[:], fl.unsqueeze(2).to_broadcast(
                    [P, ncols, ntap]), iota_b, AL.subtract)
                v.tensor_scalar(d2[:], d2[:], 0.0, 1.0, AL.max, AL.min)
                v.tensor_sub(ov[:], ov[:], d2[:])
                return ov

            def body():
                v = nc.vector
                x = xpool.tile([P, NPP], f32, tag="x")
                y = xpool.tile([P, NPP], f32, tag="y")
                sx = xpool.tile([P, NPP], f32, tag="sx")
                sy = xpool.tile([P, NPP], f32, tag="sy")
                by = xpool.tile([P, NPP], f32, tag="by")
                nc.sync.dma_start(bxc[:], bx_in[:].rearrange("(p c) -> p c",
                                                             c=1))
                nc.sync.dma_start(x[:], x_t)
                nc.sync.dma_start(y[:], y_t)
                nc.sync.dma_start(sx[:], sx_t)
                nc.sync.dma_start(sy[:], sy_t)
                nc.sync.dma_start(by[:], by_t)

                tmp = spool.tile([P, NPP], f32, tag="tmp")

                def axis_prep(pos, size, inv_bs, base_b, tag, n=NPP):
                    """fl = pos/bs - base, fh = (pos+size)/bs - base (fp16)."""
                    fl = gpool.tile([P, n], f16, tag=f"{tag}fl")
                    fh = gpool.tile([P, n], f16, tag=f"{tag}fh")
                    v.scalar_tensor_tensor(out=fl[:], in0=pos,
                                           scalar=inv_bs, in1=base_b,
                                           op0=AL.mult, op1=AL.subtract)
                    t = tmp[:, 0:n]
                    v.tensor_tensor(t, pos, size, AL.add)
                    v.scalar_tensor_tensor(out=fh[:], in0=t,
                                           scalar=inv_bs, in1=base_b,
                                           op0=AL.mult, op1=AL.subtract)
                    return fl, fh

                bx_b = bxc[:].to_broadcast([P, NPP])
                flx, fhx = axis_prep(x[:], sx[:], INV_BSX, bx_b, "x")
                fly, fhy = axis_prep(y[:], sy[:], INV_BSY, by[:], "y")

                CC = CHY * KCAP          # main slot cols per chunk

                area = opool.tile([P, NPPO], f32, tag="area")
                for ch in range(NCHUNK):
                    tch = tpool.tile([P, CHY * ESIZE], f16, tag="t")
                    nc.sync.dma_start(
                        tch[:], wt_t[:, ch * CHY * ESIZE:
                                     (ch + 1) * CHY * ESIZE])
                    t4 = tch[:].rearrange("p (c b a) -> p c b a", b=NTAPY,
                                          a=NTAPX)
                    cs = slice(ch * CC, (ch + 1) * CC)
                    ovx = weights(v, flx[:, cs], fhx[:, cs], CC, NTAPX,
                                  wpool, "wx")
                    ovy = weights(v, fly[:, cs], fhy[:, cs], CC, NTAPY,
                                  wpool, "wy")
                    for j in range(KCAP):
                        # slot columns c = KCAP*c1 + j within this chunk
                        ovx_j = bass.AP(
                            ovx[:].tensor, ovx[:].offset + j * NTAPX,
                            [ovx[:].ap[0], [KCAP * NTAPX, CHY],
                             [0, NTAPY], [1, NTAPX]])
                        ovy_j = bass.AP(
                            ovy[:].tensor, ovy[:].offset + j * NTAPY,
                            [ovy[:].ap[0], [KCAP * NTAPY, CHY],
                             [1, NTAPY]])
                        m = rpool.tile([P, CHY, NTAPY, NTAPX], f16, tag="m")
                        t2 = rpool.tile([P, CHY, NTAPY], f16, tag="t2")
                        v.tensor_tensor(m[:], t4, ovx_j, AL.mult)
                        with nc.allow_low_precision(
                                reason="fp16 7-tap partials; tol is 2e-2"):
                            v.tensor_reduce(t2[:], m[:], AX.X, AL.add)
                        v.tensor_tensor(t2[:], t2[:], ovy_j, AL.mult)
                        c0 = ch * CC + j
                        a_j = bass.AP(
                            area[:].tensor, area[:].offset + c0,
                            [area[:].ap[0], [KCAP, CHY], [1, 1]])
                        v.tensor_reduce(a_j, t2[:], AX.X, AL.add)

                # ---- overflow tier: host-embedded records ----
                ovin = xpool.tile([P, OVC * 6], f32, tag="ovin")
                orec = xpool.tile([P, OVC * ESIZE], f16, tag="orec")
                nc.sync.dma_start(ovin[:], ov_t)
                nc.sync.dma_start(orec[:], orec_t)
                ox = ovin[:, 0 * OVC:1 * OVC]
                oy = ovin[:, 1 * OVC:2 * OVC]
                osx = ovin[:, 2 * OVC:3 * OVC]
                osy = ovin[:, 3 * OVC:4 * OVC]
                obx = ovin[:, 4 * OVC:5 * OVC]
                oby = ovin[:, 5 * OVC:6 * OVC]
                flo, fho = axis_prep(ox, osx, INV_BSX, obx, "ox", n=OVC)
                flo2, fho2 = axis_prep(oy, osy, INV_BSY, oby, "oy", n=OVC)
                ovxo = weights(v, flo[:], fho[:], OVC, NTAPX, wpool, "ox")
                ovyo = weights(v, flo2[:], fho2[:], OVC, NTAPY, wpool, "oy")
                r4 = orec[:].rearrange("p (c b a) -> p c b a", b=NTAPY,
                                       a=NTAPX)
                mo = rpool.tile([P, OVC, NTAPY, NTAPX], f16, tag="mo")
                t2o = rpool.tile([P, OVC, NTAPY], f16, tag="t2o")
                v.tensor_tensor(mo[:], r4,
                                ovxo[:].unsqueeze(2).to_broadcast(
                                    [P, OVC, NTAPY, NTAPX]), AL.mult)
                with nc.allow_low_precision(
                        reason="fp16 7-tap partials; tol is 2e-2"):
                    v.tensor_reduce(t2o[:], mo[:], AX.X, AL.add)
                v.tensor_tensor(t2o[:], t2o[:], ovyo[:], AL.mult)
                v.tensor_reduce(area[:, NPP:NPPO].unsqueeze(2),
                                t2o[:], AX.X, AL.add)

                nc.sync.dma_start(out_t, area[:])

            if repeat == 1:
                body()
            else:
                with tc.For_i(0, repeat, 1):
                    body()

    nc.compile()
    return nc


def make_table(utilization_map):
    """WT[r, b, a] = U[4*(r>>8) + a, 2*(r&255) + b] * BSX*BSY, fp16.
    a in 0..6 (7th zero-padded), b in 0..4; edges zero-padded."""
    U = np.asarray(utilization_map, np.float32) * np.float32(BSX * BSY)
    Upad = np.zeros((512 + NTAPX, 512 + NTAPY), np.float32)
    Upad[:512, :512] = U
    qx = np.arange(P)
    hy = np.arange(NHY)
    a = np.arange(NTAPX)
    b = np.arange(NTAPY)
    rows = 4 * qx[:, None, None, None] + a[None, None, None, :]
    cols = 2 * hy[None, :, None, None] + b[None, None, :, None]
    win = Upad[rows, cols]                       # [128, 256, 5(b), 8(a)]
    win[:, :, :, 7] = 0.0
    return win.astype(np.float16)                # [P, NHY, 5, 8]


def prepare(pos, node_size_x, node_size_y, utilization_map):
    """Bucket nodes into (core, output slot); return per-core input maps
    plus each node's (core, flat output index) for unsharding."""
    n = NUM_MOVABLE
    half = pos.shape[0] // 2
    x = np.asarray(pos[:n], np.float32)
    y = np.asarray(pos[half:half + n], np.float32)
    sx = np.asarray(node_size_x, np.float32)
    sy = np.asarray(node_size_y, np.float32)

    # window base per node, matching the reference's f32 chain
    bx0 = np.clip(np.floor(x / np.float32(BSX)).astype(np.int32), 0, NBX - 1)
    by0 = np.clip(np.floor(y / np.float32(BSY)).astype(np.int32), 0, NBY - 1)
    qx = (bx0 >> 2).astype(np.int64)
    hy = (by0 >> 1).astype(np.int64)
    rec = qx * NHY + hy

    order = np.argsort(rec, kind="stable")
    rs = rec[order]
    starts = np.flatnonzero(np.r_[True, np.diff(rs) != 0])
    run_id = np.cumsum(np.r_[0, (np.diff(rs) != 0).astype(np.int64)])
    pos_in_rec = np.arange(n, dtype=np.int64) - starts[run_id]
    core = pos_in_rec % NCORES
    k = pos_in_rec // NCORES
    # overflow nodes carry their record explicitly, so their core choice is
    # free — deal them globally round-robin for balance (per-record dealing
    # would pile them all onto low cores: pos 32 -> core 0, 33 -> 1, ...)
    ovsel = k >= KCAP
    core[ovsel] = np.arange(int(ovsel.sum()), dtype=np.int64) % NCORES

    wt = make_table(utilization_map)             # [P, NHY, 5, 8] fp16
    wt2d = wt.reshape(NREC, ESIZE)
    bx = (4.0 * np.arange(P)).astype(np.float32)
    by = np.broadcast_to(
        2.0 * (np.arange(NPP, dtype=np.float32) // KCAP),
        (P, NPP)).astype(np.float32).reshape(-1)

    main = k < KCAP
    slot = rs * KCAP + k                         # main slot id in the core
    node_core = np.empty(n, np.int64)
    node_out = np.empty(n, np.int64)             # flat output index
    node_core[order] = core
    # main: output index p*NPPO + (slot % NPP)
    node_out[order[main]] = ((slot[main] // NPP) * NPPO + slot[main] % NPP)

    in_maps = []
    ovstats = []
    for c in range(NCORES):
        mc = core == c
        mcm = mc & main
        s = slot[mcm]
        idx = order[mcm]
        xp = np.zeros(NPC, np.float32)
        yp = np.zeros(NPC, np.float32)
        sxp = np.zeros(NPC, np.float32)
        syp = np.zeros(NPC, np.float32)
        xp[s] = x[idx]
        yp[s] = y[idx]
        sxp[s] = sx[idx]
        syp[s] = sy[idx]

        # overflow tier
        mco = mc & ~main
        oidx = order[mco]
        nov = oidx.size
        assert nov <= NOV, f"overflow {nov} exceeds capacity {NOV}"
        ovstats.append(nov)
        ovr = rs[mco]
        ovp = np.zeros((6, P, OVC), np.float32)
        orec = np.zeros((P, OVC, ESIZE), np.float16)
        op_ = np.arange(nov) // OVC
        oc_ = np.arange(nov) % OVC
        ovp[0, op_, oc_] = x[oidx]
        ovp[1, op_, oc_] = y[oidx]
        ovp[2, op_, oc_] = sx[oidx]
        ovp[3, op_, oc_] = sy[oidx]
        ovp[4, op_, oc_] = 4.0 * (ovr // NHY)
        ovp[5, op_, oc_] = 2.0 * (ovr % NHY)
        orec[op_, oc_] = wt2d[ovr]
        node_out[oidx] = op_ * NPPO + NPP + oc_

        in_maps.append(dict(
            x_in=xp, y_in=yp, sx_in=sxp, sy_in=syp,
            wt_in=wt.reshape(-1), bx_in=bx, by_in=by,
            ov_in=ovp.transpose(1, 0, 2).reshape(-1),
            orec_in=orec.reshape(-1)))
    return in_maps, (node_core, node_out)


def unshard(outs, meta):
    """outs: per-core [P*NPPO] slot-area arrays -> [N] node areas."""
    node_core, node_out = meta
    stacked = np.stack([np.asarray(o).reshape(-1) for o in outs])
    return stacked[node_core, node_out].astype(np.float32)


_NC_CACHE = {}


def _get_nc(repeat=1):
    if repeat not in _NC_CACHE:
        _NC_CACHE[repeat] = build(repeat)
    return _NC_CACHE[repeat]


def kernel(pos, node_size_x, node_size_y, utilization_map):
    in_maps, meta = prepare(pos, node_size_x, node_size_y, utilization_map)
    nc = _get_nc(1)
    res = bass_utils.run_bass_kernel_spmd(nc, in_maps,
                                          core_ids=list(range(NCORES)))
    return unshard([r["area_out"] for r in res.results], meta)
